# revision 6
# baseline (speedup 1.0000x reference)
"""Multi-head attention (B=2, T=2048, D=1024, H=16) on 8 TRN2 NeuronCores.

Sharding: 2D (batch x head-group). Core c handles batch b = c // 4 and head
group hg = c % 4 (4 heads = 256 channels of the projected dim). Each core:
  1. Projects its batch's q/k/v against its 256-row weight slices -> QT/KT
     in [j, t] layout and V in [t, j] layout (bf16, fp32 PSUM accumulation).
  2. Per head: S.T = K_h @ Q_h.T (scores transposed), U = exp(S.T * scale)
     (no max subtraction: |S*scale| <= ~16, exp fits fp32 easily),
     O.T += V_h.T @ U, denom += ones.T @ U (softmax denominator via PE).
  3. Normalizes O.T by 1/denom (broadcast via a small selector matmul).
  4. out_partial.T = woT_chunk.T @ O_norm.T  -> [1024, 2048] fp32.
Host sums the 4 head-group partials per batch, transposes, adds bo.

PSUM discipline: exactly one accumulation group per PSUM bank (hardware
start=True clears has_written bits bank-wide, so interleaved groups in a
bank are unsafe). Head-pair concurrency comes from tile_position row/col
groups with outputs in separate banks.

All shapes are hardcoded for this problem. kernel() takes the full inputs
and returns the full [2, 2048, 1024] fp32 output.
"""

import numpy as np
import ml_dtypes

import concourse.bass as bass
import concourse.bacc as bacc
import concourse.mybir as mybir
import concourse.tile as tile
from concourse.bass_utils import run_bass_kernel_spmd

B, T, D, H, Hd = 2, 2048, 1024, 16, 64
HPC = 4          # heads per core
W = HPC * Hd     # 256 projected channels per core
SCALE = Hd ** -0.5
N_CORES = 8

BF16 = mybir.dt.bfloat16
F32 = mybir.dt.float32
bf16 = ml_dtypes.bfloat16


def build_nc():
    nc = bacc.Bacc("TRN2", target_bir_lowering=False, debug=False)

    xq = nc.dram_tensor("xq", [D, T], BF16, kind="ExternalInput").ap()
    xk = nc.dram_tensor("xk", [D, T], BF16, kind="ExternalInput").ap()
    xv = nc.dram_tensor("xv", [D, T], BF16, kind="ExternalInput").ap()
    wq = nc.dram_tensor("wq", [D, W], BF16, kind="ExternalInput").ap()
    wk = nc.dram_tensor("wk", [D, W], BF16, kind="ExternalInput").ap()
    wv = nc.dram_tensor("wv", [D, W], BF16, kind="ExternalInput").ap()
    wo = nc.dram_tensor("wo", [W, D], BF16, kind="ExternalInput").ap()
    bq = nc.dram_tensor("bq", [1, W], BF16, kind="ExternalInput").ap()
    bk = nc.dram_tensor("bk", [1, W], BF16, kind="ExternalInput").ap()
    bv = nc.dram_tensor("bv", [1, W], BF16, kind="ExternalInput").ap()
    out = nc.dram_tensor("out", [D, T], F32, kind="ExternalOutput").ap()

    Exp = mybir.ActivationFunctionType.Exp

    with tile.TileContext(nc) as tc:
        with (
            tc.tile_pool(name="persist", bufs=1) as persist,
            tc.tile_pool(name="xpool", bufs=2) as xpool,
            tc.tile_pool(name="upool", bufs=3) as upool,
            tc.tile_pool(name="rpool", bufs=2) as rpool,
            tc.tile_pool(name="opool", bufs=2) as opool,
        ):
            # ---- constants ----
            ones_col = persist.tile([128, 1], BF16, tag="ones_col")
            nc.vector.memset(ones_col, 1.0)
            ones_row = persist.tile([1, 512], BF16, tag="ones_row")
            nc.vector.memset(ones_row, 1.0)
            ones128 = persist.tile([1, 128], BF16, tag="ones128")
            nc.vector.memset(ones128, 1.0)
            # selector for denominator broadcast: row 64 -> out rows 0-63
            # (head m=0 recip lives at partition 64), row 0 -> out rows 64-127
            # (head m=1 recip lives at partition 0).
            sel = persist.tile([65, 128], F32, tag="sel")
            nc.vector.memset(sel, 0.0)
            nc.vector.memset(sel[64:65, 0:64], 1.0)
            nc.vector.memset(sel[0:1, 64:128], 1.0)

            # ---- weights / biases ----
            wq_sb = persist.tile([128, 8, W], BF16, tag="wq")
            nc.sync.dma_start(out=wq_sb, in_=wq.rearrange("(c p) j -> p c j", p=128))
            wk_sb = persist.tile([128, 8, W], BF16, tag="wk")
            nc.sync.dma_start(out=wk_sb, in_=wk.rearrange("(c p) j -> p c j", p=128))
            wv_sb = persist.tile([128, 8, W], BF16, tag="wv")
            nc.sync.dma_start(out=wv_sb, in_=wv.rearrange("(c p) j -> p c j", p=128))
            wo_sb = persist.tile([128, 2, D], BF16, tag="wo")
            nc.sync.dma_start(out=wo_sb, in_=wo.rearrange("(c p) e -> p c e", p=128))
            bq_sb = persist.tile([1, W], BF16, tag="bq")
            nc.sync.dma_start(out=bq_sb, in_=bq)
            bk_sb = persist.tile([1, W], BF16, tag="bk")
            nc.sync.dma_start(out=bk_sb, in_=bk)
            bv_sb = persist.tile([1, W], BF16, tag="bv")
            nc.sync.dma_start(out=bv_sb, in_=bv)

            # ---- persistent activations ----
            qt_sb = persist.tile([128, 2, T], BF16, tag="qt")   # QT [j, t]
            kt_sb = persist.tile([128, 2, T], BF16, tag="kt")   # KT [j, t]
            v_sb = persist.tile([128, 16, W], BF16, tag="v")    # V  [t, j] by k-tile
            otn_sb = persist.tile([128, 2, T], BF16, tag="otn")  # normalized O.T

            # ================= Phase A: projections =================
            with tc.tile_pool(name="psA", bufs=8, space="PSUM") as psA:
                # --- Q and K -> [j, t] layout ---
                for x_dram, w_sb, b_sb, dst in (
                    (xq, wq_sb, bq_sb, qt_sb),
                    (xk, wk_sb, bk_sb, kt_sb),
                ):
                    ps = [psA.tile([128, 512], F32, tag="proj", name=f"proj{i}")
                          for i in range(8)]
                    for c in range(8):
                        xc = xpool.tile([128, T], BF16, tag="x")
                        nc.gpsimd.dma_start(out=xc, in_=x_dram[c * 128:(c + 1) * 128, :])
                        for jt in range(2):
                            for tt in range(4):
                                nc.tensor.matmul(
                                    ps[jt * 4 + tt],
                                    lhsT=w_sb[:, c, jt * 128:(jt + 1) * 128],
                                    rhs=xc[:, tt * 512:(tt + 1) * 512],
                                    start=(c == 0), stop=False,
                                )
                    for jt in range(2):
                        for tt in range(4):
                            p = ps[jt * 4 + tt]
                            nc.tensor.matmul(
                                p, lhsT=b_sb[:, jt * 128:(jt + 1) * 128],
                                rhs=ones_row, start=False, stop=True,
                            )
                            nc.vector.tensor_copy(
                                dst[:, jt, tt * 512:(tt + 1) * 512], p)

                # --- V -> [t, j] layout (2 waves of 8 t-tiles) ---
                for w in range(2):
                    ps = [psA.tile([128, W], F32, tag="proj", name=f"proj{i}")
                          for i in range(8)]
                    for c in range(8):
                        xc = xpool.tile([128, T], BF16, tag="x")
                        nc.gpsimd.dma_start(out=xc, in_=xv[c * 128:(c + 1) * 128, :])
                        for i in range(8):
                            tt = w * 8 + i
                            nc.tensor.matmul(
                                ps[i],
                                lhsT=xc[:, tt * 128:(tt + 1) * 128],
                                rhs=wv_sb[:, c, :],
                                start=(c == 0), stop=False,
                            )
                    for i in range(8):
                        tt = w * 8 + i
                        nc.tensor.matmul(ps[i], lhsT=ones128, rhs=bv_sb,
                                         start=False, stop=True)
                        nc.vector.tensor_copy(v_sb[:, tt, :], ps[i])

            # ============ Phase B/D: attention per head pair ============
            with tc.tile_pool(name="psB", bufs=1, space="PSUM") as psB:
                for pr in range(2):
                    for qt in range(4):
                        qsl = slice(qt * 512, (qt + 1) * 512)
                        # one accumulation group per bank:
                        o_ps0 = psB.tile([128, 512], F32, tag="o0", bufs=1)
                        o_ps1 = psB.tile([128, 512], F32, tag="o1", bufs=1)
                        d_ps0 = psB.tile([65, 512], F32, tag="d0", bufs=1)
                        d_ps1 = psB.tile([1, 512], F32, tag="d1", bufs=1)
                        for kt in range(16):
                            us = []
                            for m in range(2):
                                po = 64 * m
                                s_ps = psB.tile([128, 512], F32, tag="s", bufs=2,
                                                name=f"s{m}")
                                nc.tensor.matmul(
                                    s_ps,
                                    lhsT=kt_sb[po:po + 64, pr, kt * 128:(kt + 1) * 128],
                                    rhs=qt_sb[po:po + 64, pr, qsl],
                                    start=True, stop=True,
                                )
                                u = upool.tile([128, 512], BF16, tag="u",
                                               name=f"u{m}")
                                nc.scalar.activation(u, s_ps, Exp, scale=SCALE)
                                us.append(u)
                            h0, h1 = 2 * pr, 2 * pr + 1
                            last = (kt == 15)
                            first = (kt == 0)
                            # pass 1: O.T head m=0 (array cols 0-63) + its
                            # denominator (array col 64, own bank, row 64).
                            nc.tensor.matmul(
                                o_ps0[0:64, :],
                                lhsT=v_sb[:, kt, h0 * 64:(h0 + 1) * 64],
                                rhs=us[0],
                                start=first, stop=last, tile_position=(0, 0),
                            )
                            nc.tensor.matmul(
                                d_ps0[64:65, :],
                                lhsT=ones_col, rhs=us[0],
                                start=first, stop=last, tile_position=(0, 64),
                            )
                            # pass 2: O.T head m=1 (array cols 64-127) + its
                            # denominator (array col 0, own bank, row 0).
                            nc.tensor.matmul(
                                o_ps1[64:128, :],
                                lhsT=v_sb[:, kt, h1 * 64:(h1 + 1) * 64],
                                rhs=us[1],
                                start=first, stop=last, tile_position=(0, 64),
                            )
                            nc.tensor.matmul(
                                d_ps1[0:1, :],
                                lhsT=ones_col, rhs=us[1],
                                start=first, stop=last, tile_position=(0, 0),
                            )
                        # normalization: rb[j, q] = 1/denom(head(j))[q]
                        den_sb = rpool.tile([65, 512], F32, tag="den")
                        nc.vector.memset(den_sb, 1.0)
                        nc.vector.tensor_copy(den_sb[64:65, :], d_ps0[64:65, :])
                        nc.vector.tensor_copy(den_sb[0:1, :], d_ps1)
                        rsb = rpool.tile([65, 512], F32, tag="recip")
                        nc.vector.reciprocal(rsb, den_sb)
                        rb_ps = psB.tile([128, 512], F32, tag="rb", bufs=1)
                        nc.tensor.matmul(rb_ps, lhsT=sel, rhs=rsb,
                                         start=True, stop=True)
                        rb_sb = rpool.tile([128, 512], F32, tag="rbs")
                        nc.vector.tensor_copy(rb_sb, rb_ps)
                        nc.vector.tensor_mul(
                            otn_sb[0:64, pr, qsl], o_ps0[0:64, :], rb_sb[0:64, :])
                        nc.vector.tensor_mul(
                            otn_sb[64:128, pr, qsl], o_ps1[64:128, :],
                            rb_sb[64:128, :])

            # ================= Phase E: output projection =================
            with tc.tile_pool(name="psE", bufs=4, space="PSUM") as psE:
                for et in range(8):
                    stg = opool.tile([128, T], F32, tag="ostg")
                    for tt in range(4):
                        tsl = slice(tt * 512, (tt + 1) * 512)
                        e_ps = psE.tile([128, 512], F32, tag="e")
                        for jc in range(2):
                            nc.tensor.matmul(
                                e_ps,
                                lhsT=wo_sb[:, jc, et * 128:(et + 1) * 128],
                                rhs=otn_sb[:, jc, tsl],
                                start=(jc == 0), stop=(jc == 1),
                            )
                        nc.vector.tensor_copy(stg[:, tsl], e_ps)
                    nc.sync.dma_start(out=out[et * 128:(et + 1) * 128, :], in_=stg)

    nc.finalize()
    return nc


_NC_CACHE = None


def _get_nc():
    global _NC_CACHE
    if _NC_CACHE is None:
        _NC_CACHE = build_nc()
    return _NC_CACHE


def make_in_maps(query, key, value, wq, bq, wk, bk, wv, bv, wo, bo):
    in_maps = []
    for c in range(N_CORES):
        b, hg = divmod(c, HPC)
        sl = slice(hg * W, (hg + 1) * W)
        in_maps.append({
            "xq": np.ascontiguousarray(np.asarray(query[b]).T).astype(bf16),
            "xk": np.ascontiguousarray(np.asarray(key[b]).T).astype(bf16),
            "xv": np.ascontiguousarray(np.asarray(value[b]).T).astype(bf16),
            "wq": np.ascontiguousarray(np.asarray(wq)[sl].T).astype(bf16),
            "wk": np.ascontiguousarray(np.asarray(wk)[sl].T).astype(bf16),
            "wv": np.ascontiguousarray(np.asarray(wv)[sl].T).astype(bf16),
            "wo": np.ascontiguousarray(np.asarray(wo)[:, sl].T).astype(bf16),
            "bq": np.asarray(bq)[sl].reshape(1, W).astype(bf16),
            "bk": np.asarray(bk)[sl].reshape(1, W).astype(bf16),
            "bv": np.asarray(bv)[sl].reshape(1, W).astype(bf16),
        })
    return in_maps


def combine_outputs(outs, bo):
    full = np.zeros((B, T, D), np.float32)
    for c in range(N_CORES):
        b = c // HPC
        full[b] += outs[c].T
    full += np.asarray(bo, np.float32)[None, None, :]
    return full


def kernel(query, key, value, wq, bq, wk, bk, wv, bv, wo, bo):
    nc = _get_nc()
    in_maps = make_in_maps(query, key, value, wq, bq, wk, bk, wv, bv, wo, bo)
    res = run_bass_kernel_spmd(nc, in_maps, list(range(N_CORES)))
    outs = [np.asarray(res.results[c]["out"]) for c in range(N_CORES)]
    return combine_outputs(outs, bo)


# revision 8
# speedup vs baseline: 1.1586x; 1.1586x over previous
"""Multi-head attention (B=2, T=2048, D=1024, H=16) on 8 TRN2 NeuronCores.

Sharding: 2D (batch x head-group). Core c handles batch b = c // 4 and head
group hg = c % 4 (4 heads = 256 channels of the projected dim). Each core:
  1. Projects its batch's q/k/v against its 256-row weight slices -> QT/KT
     in [j, t] layout and V in [t, j] layout (bf16, fp32 PSUM accumulation).
     V is stored augmented with a ones column per head: [V_h | 1].
  2. Per head: S.T = K_h @ Q_h.T (scores transposed), U = exp(S.T * scale)
     (no max subtraction: |S*scale| <= ~16, exp fits fp32 easily), then
     [O.T ; denom] += [V_h | 1].T @ U -- the softmax denominator rides the
     PV matmul for free as output row 64.
  3. Normalizes O.T by 1/denom (broadcast via small selector matmuls).
  4. out_partial.T = woT_chunk.T @ O_norm.T  -> [1024, 2048] fp32.
Host sums the 4 head-group partials per batch, transposes, adds bo.

PSUM discipline: exactly one accumulation group per PSUM bank (hardware
start=True clears has_written bits bank-wide, so interleaved groups in a
bank are unsafe). Partition shifts (head m=1 belongs at rows 64-127 of the
stage-E operand but PSUM results are at rows 0-64) use small SBUF->SBUF
DMAs, since compute engines cannot cross partitions.

All shapes are hardcoded for this problem. kernel() takes the full inputs
and returns the full [2, 2048, 1024] fp32 output.
"""

import numpy as np
import ml_dtypes

import concourse.bass as bass
import concourse.bacc as bacc
import concourse.mybir as mybir
import concourse.tile as tile
from concourse.bass_utils import run_bass_kernel_spmd

B, T, D, H, Hd = 2, 2048, 1024, 16, 64
HPC = 4          # heads per core
W = HPC * Hd     # 256 projected channels per core
SCALE = Hd ** -0.5
N_CORES = 8

BF16 = mybir.dt.bfloat16
F32 = mybir.dt.float32
bf16 = ml_dtypes.bfloat16


def build_nc():
    nc = bacc.Bacc("TRN2", target_bir_lowering=False, debug=False)

    xq = nc.dram_tensor("xq", [D, T], BF16, kind="ExternalInput").ap()
    xk = nc.dram_tensor("xk", [D, T], BF16, kind="ExternalInput").ap()
    xv = nc.dram_tensor("xv", [D, T], BF16, kind="ExternalInput").ap()
    wq = nc.dram_tensor("wq", [D, W], BF16, kind="ExternalInput").ap()
    wk = nc.dram_tensor("wk", [D, W], BF16, kind="ExternalInput").ap()
    wv = nc.dram_tensor("wv", [D, W], BF16, kind="ExternalInput").ap()
    wo = nc.dram_tensor("wo", [W, D], BF16, kind="ExternalInput").ap()
    bq = nc.dram_tensor("bq", [1, W], BF16, kind="ExternalInput").ap()
    bk = nc.dram_tensor("bk", [1, W], BF16, kind="ExternalInput").ap()
    bv = nc.dram_tensor("bv", [1, W], BF16, kind="ExternalInput").ap()
    out = nc.dram_tensor("out", [D, T], F32, kind="ExternalOutput").ap()

    Exp = mybir.ActivationFunctionType.Exp

    with tile.TileContext(nc) as tc:
        with (
            tc.tile_pool(name="persist", bufs=1) as persist,
            tc.tile_pool(name="xpool", bufs=2) as xpool,
            tc.tile_pool(name="upool", bufs=4) as upool,
            tc.tile_pool(name="rpool", bufs=2) as rpool,
            tc.tile_pool(name="opool", bufs=2) as opool,
        ):
            # ---- constants ----
            ones_row = persist.tile([1, 512], BF16, tag="ones_row")
            nc.vector.memset(ones_row, 1.0)
            ones128 = persist.tile([1, 128], BF16, tag="ones128")
            nc.vector.memset(ones128, 1.0)
            # selector matmuls broadcasting one recip row to 64 partitions:
            # selA: row 64 -> all 64 out rows; selB: row 0 -> all 64 out rows.
            selA = persist.tile([65, 64], F32, tag="selA")
            nc.vector.memset(selA, 0.0)
            nc.vector.memset(selA[64:65, :], 1.0)
            selB = persist.tile([65, 64], F32, tag="selB")
            nc.vector.memset(selB, 0.0)
            nc.vector.memset(selB[0:1, :], 1.0)

            # ---- weights / biases ----
            wq_sb = persist.tile([128, 8, W], BF16, tag="wq")
            nc.sync.dma_start(out=wq_sb, in_=wq.rearrange("(c p) j -> p c j", p=128))
            wk_sb = persist.tile([128, 8, W], BF16, tag="wk")
            nc.sync.dma_start(out=wk_sb, in_=wk.rearrange("(c p) j -> p c j", p=128))
            wv_sb = persist.tile([128, 8, W], BF16, tag="wv")
            nc.sync.dma_start(out=wv_sb, in_=wv.rearrange("(c p) j -> p c j", p=128))
            wo_sb = persist.tile([128, 2, D], BF16, tag="wo")
            nc.sync.dma_start(out=wo_sb, in_=wo.rearrange("(c p) e -> p c e", p=128))
            bq_sb = persist.tile([1, W], BF16, tag="bq")
            nc.sync.dma_start(out=bq_sb, in_=bq)
            bk_sb = persist.tile([1, W], BF16, tag="bk")
            nc.sync.dma_start(out=bk_sb, in_=bk)
            bv_sb = persist.tile([1, W], BF16, tag="bv")
            nc.sync.dma_start(out=bv_sb, in_=bv)

            # ---- persistent activations ----
            qt_sb = persist.tile([128, 2, T], BF16, tag="qt")   # QT [j, t]
            kt_sb = persist.tile([128, 2, T], BF16, tag="kt")   # KT [j, t]
            # V augmented with ones column per head: [k, kt, h, 0:64]=V, [..64]=1
            vaug_sb = persist.tile([128, 16, HPC, Hd + 1], BF16, tag="vaug")
            nc.vector.memset(vaug_sb[:, :, :, 64:65], 1.0)
            otn_sb = persist.tile([128, 2, T], BF16, tag="otn")  # normalized O.T

            # ================= Phase A: projections =================
            with tc.tile_pool(name="psA", bufs=8, space="PSUM") as psA:
                # --- Q and K -> [j, t] layout ---
                for x_dram, w_sb, b_sb, dst in (
                    (xq, wq_sb, bq_sb, qt_sb),
                    (xk, wk_sb, bk_sb, kt_sb),
                ):
                    ps = [psA.tile([128, 512], F32, tag="proj", name=f"proj{i}")
                          for i in range(8)]
                    for c in range(8):
                        xc = xpool.tile([128, T], BF16, tag="x")
                        nc.gpsimd.dma_start(out=xc, in_=x_dram[c * 128:(c + 1) * 128, :])
                        for jt in range(2):
                            for tt in range(4):
                                nc.tensor.matmul(
                                    ps[jt * 4 + tt],
                                    lhsT=w_sb[:, c, jt * 128:(jt + 1) * 128],
                                    rhs=xc[:, tt * 512:(tt + 1) * 512],
                                    start=(c == 0), stop=False,
                                )
                    for jt in range(2):
                        for tt in range(4):
                            p = ps[jt * 4 + tt]
                            nc.tensor.matmul(
                                p, lhsT=b_sb[:, jt * 128:(jt + 1) * 128],
                                rhs=ones_row, start=False, stop=True,
                            )
                            nc.vector.tensor_copy(
                                dst[:, jt, tt * 512:(tt + 1) * 512], p)

                # --- V -> [t, j] layout (2 waves of 8 t-tiles) ---
                for w in range(2):
                    ps = [psA.tile([128, W], F32, tag="proj", name=f"proj{i}")
                          for i in range(8)]
                    for c in range(8):
                        xc = xpool.tile([128, T], BF16, tag="x")
                        nc.gpsimd.dma_start(out=xc, in_=xv[c * 128:(c + 1) * 128, :])
                        for i in range(8):
                            tt = w * 8 + i
                            nc.tensor.matmul(
                                ps[i],
                                lhsT=xc[:, tt * 128:(tt + 1) * 128],
                                rhs=wv_sb[:, c, :],
                                start=(c == 0), stop=False,
                            )
                    for i in range(8):
                        tt = w * 8 + i
                        nc.tensor.matmul(ps[i], lhsT=ones128, rhs=bv_sb,
                                         start=False, stop=True)
                        nc.vector.tensor_copy(
                            vaug_sb[:, tt, :, 0:64],
                            ps[i].rearrange("p (h d) -> p h d", h=HPC))

            # ============ Phase B/D: attention per head pair ============
            with tc.tile_pool(name="psB", bufs=1, space="PSUM") as psB:
                for pr in range(2):
                    for qt in range(4):
                        qsl = slice(qt * 512, (qt + 1) * 512)
                        # [O.T ; denom] accumulators, one bank + one group each
                        o_psA = psB.tile([65, 512], F32, tag="oA", bufs=2)
                        o_psB = psB.tile([65, 512], F32, tag="oB", bufs=2)
                        for kt in range(16):
                            for m, o_ps in ((0, o_psA), (1, o_psB)):
                                h = 2 * pr + m
                                po = 64 * m
                                s_ps = psB.tile([128, 512], F32, tag="s", bufs=3,
                                                name=f"s{m}")
                                nc.tensor.matmul(
                                    s_ps,
                                    lhsT=kt_sb[po:po + 64, pr, kt * 128:(kt + 1) * 128],
                                    rhs=qt_sb[po:po + 64, pr, qsl],
                                    start=True, stop=True,
                                )
                                u = upool.tile([128, 512], BF16, tag="u",
                                               name=f"u{m}")
                                nc.scalar.activation(u, s_ps, Exp, scale=SCALE)
                                nc.tensor.matmul(
                                    o_ps,
                                    lhsT=vaug_sb[:, kt, h, :],
                                    rhs=u,
                                    start=(kt == 0), stop=(kt == 15),
                                )
                        # ---- normalization ----
                        # den rows: m0 -> row 64 (aligned copy); m1 -> row 0
                        # (via sbuf->sbuf DMA partition shift).
                        den_sb = rpool.tile([65, 512], F32, tag="den")
                        nc.vector.memset(den_sb, 1.0)
                        tmpd = rpool.tile([65, 512], F32, tag="tmpd")
                        nc.vector.tensor_copy(den_sb[64:65, :], o_psA[64:65, :])
                        nc.vector.tensor_copy(tmpd[64:65, :], o_psB[64:65, :])
                        nc.sync.dma_start(out=den_sb[0:1, :], in_=tmpd[64:65, :])
                        rsb = rpool.tile([65, 512], F32, tag="recip")
                        nc.vector.reciprocal(rsb, den_sb)
                        # broadcast each head's recip row to 64 partitions
                        rbA_ps = psB.tile([64, 512], F32, tag="s", bufs=3, name="rbA")
                        nc.tensor.matmul(rbA_ps, lhsT=selA, rhs=rsb,
                                         start=True, stop=True)
                        rbB_ps = psB.tile([64, 512], F32, tag="s", bufs=3, name="rbB")
                        nc.tensor.matmul(rbB_ps, lhsT=selB, rhs=rsb,
                                         start=True, stop=True)
                        rbA_sb = rpool.tile([64, 512], F32, tag="rbAs")
                        nc.vector.tensor_copy(rbA_sb, rbA_ps)
                        rbB_sb = rpool.tile([64, 512], F32, tag="rbBs")
                        nc.vector.tensor_copy(rbB_sb, rbB_ps)
                        # head m=0 -> otn rows 0-63 directly
                        nc.vector.tensor_mul(
                            otn_sb[0:64, pr, qsl], o_psA[0:64, :], rbA_sb)
                        # head m=1 -> rows 0-63 staging, DMA-shift to 64-127
                        otnB = rpool.tile([64, 512], BF16, tag="otnB")
                        nc.vector.tensor_mul(otnB, o_psB[0:64, :], rbB_sb)
                        nc.sync.dma_start(out=otn_sb[64:128, pr, qsl], in_=otnB)

            # ================= Phase E: output projection =================
            with tc.tile_pool(name="psE", bufs=4, space="PSUM") as psE:
                for et in range(8):
                    stg = opool.tile([128, T], F32, tag="ostg")
                    for tt in range(4):
                        tsl = slice(tt * 512, (tt + 1) * 512)
                        e_ps = psE.tile([128, 512], F32, tag="e")
                        for jc in range(2):
                            nc.tensor.matmul(
                                e_ps,
                                lhsT=wo_sb[:, jc, et * 128:(et + 1) * 128],
                                rhs=otn_sb[:, jc, tsl],
                                start=(jc == 0), stop=(jc == 1),
                            )
                        nc.vector.tensor_copy(stg[:, tsl], e_ps)
                    nc.sync.dma_start(out=out[et * 128:(et + 1) * 128, :], in_=stg)

    nc.finalize()
    return nc


_NC_CACHE = None


def _get_nc():
    global _NC_CACHE
    if _NC_CACHE is None:
        _NC_CACHE = build_nc()
    return _NC_CACHE


def make_in_maps(query, key, value, wq, bq, wk, bk, wv, bv, wo, bo):
    in_maps = []
    for c in range(N_CORES):
        b, hg = divmod(c, HPC)
        sl = slice(hg * W, (hg + 1) * W)
        in_maps.append({
            "xq": np.ascontiguousarray(np.asarray(query[b]).T).astype(bf16),
            "xk": np.ascontiguousarray(np.asarray(key[b]).T).astype(bf16),
            "xv": np.ascontiguousarray(np.asarray(value[b]).T).astype(bf16),
            "wq": np.ascontiguousarray(np.asarray(wq)[sl].T).astype(bf16),
            "wk": np.ascontiguousarray(np.asarray(wk)[sl].T).astype(bf16),
            "wv": np.ascontiguousarray(np.asarray(wv)[sl].T).astype(bf16),
            "wo": np.ascontiguousarray(np.asarray(wo)[:, sl].T).astype(bf16),
            "bq": np.asarray(bq)[sl].reshape(1, W).astype(bf16),
            "bk": np.asarray(bk)[sl].reshape(1, W).astype(bf16),
            "bv": np.asarray(bv)[sl].reshape(1, W).astype(bf16),
        })
    return in_maps


def combine_outputs(outs, bo):
    full = np.zeros((B, T, D), np.float32)
    for c in range(N_CORES):
        b = c // HPC
        full[b] += outs[c].T
    full += np.asarray(bo, np.float32)[None, None, :]
    return full


def kernel(query, key, value, wq, bq, wk, bk, wv, bv, wo, bo):
    nc = _get_nc()
    in_maps = make_in_maps(query, key, value, wq, bq, wk, bk, wv, bv, wo, bo)
    res = run_bass_kernel_spmd(nc, in_maps, list(range(N_CORES)))
    outs = [np.asarray(res.results[c]["out"]) for c in range(N_CORES)]
    return combine_outputs(outs, bo)


# revision 10
# speedup vs baseline: 1.7728x; 1.5302x over previous
"""Multi-head attention (B=2, T=2048, D=1024, H=16) on 8 TRN2 NeuronCores.

Sharding: 2D (batch x head-group). Core c handles batch b = c // 4 and head
group hg = c % 4 (4 heads = 256 channels of the projected dim). Each core:
  1. Projects its batch's q/k/v against its 256-row weight slices -> QT/KT
     in [j, t] layout and V in [t, j] layout (bf16, fp32 PSUM accumulation).
     V is stored augmented with a ones column per head: [V_h | 1].
     Order Q, V, K so attention never stalls waiting for V.
  2. Per head pair, per 512-wide q tile: S.T = K_h @ Q_h.T (transposed
     scores), U = exp(S.T * scale) (no max subtraction: |S*scale| <= ~16,
     exp fits fp32 easily), then [O.T ; denom] += [V_h | 1].T @ U -- the
     softmax denominator rides the PV matmul for free as output row 64.
     The PV matmuls trail the score/exp stage by one k tile so the PE
     never waits on ScalarE (keeps the HAM clock at 2.4 GHz).
  3. Raw [O.T ; denom] is staged to SBUF; per-block reciprocals run on
     idle DVE cycles; normalization + the output projection for q tile
     qt-1 are woven into the middle of qt's blocks as PE filler.
  4. out_partial.T = woT_chunk.T @ O_norm.T  -> [1024, 2048] fp32.
Host sums the 4 head-group partials per batch, transposes, adds bo.

PSUM discipline: exactly one accumulation group per PSUM bank (hardware
start=True clears has_written bits bank-wide). Engine ops only start at
partition offsets {0, 32, 64, 96}; partition shifts (head m=1 belongs at
rows 64-127 of the stage-E operand but results sit at rows 0-64) use
small SBUF->SBUF DMAs.

All shapes are hardcoded for this problem. kernel() takes the full inputs
and returns the full [2, 2048, 1024] fp32 output.
"""

import numpy as np
import ml_dtypes

import concourse.bass as bass
import concourse.bacc as bacc
import concourse.mybir as mybir
import concourse.tile as tile
from concourse.bass_utils import run_bass_kernel_spmd

B, T, D, H, Hd = 2, 2048, 1024, 16, 64
HPC = 4          # heads per core
W = HPC * Hd     # 256 projected channels per core
SCALE = Hd ** -0.5
N_CORES = 8

BF16 = mybir.dt.bfloat16
F32 = mybir.dt.float32
bf16 = ml_dtypes.bfloat16


def build_nc():
    nc = bacc.Bacc("TRN2", target_bir_lowering=False, debug=False)

    xq = nc.dram_tensor("xq", [D, T], BF16, kind="ExternalInput").ap()
    xk = nc.dram_tensor("xk", [D, T], BF16, kind="ExternalInput").ap()
    xv = nc.dram_tensor("xv", [D, T], BF16, kind="ExternalInput").ap()
    wq = nc.dram_tensor("wq", [D, W], BF16, kind="ExternalInput").ap()
    wk = nc.dram_tensor("wk", [D, W], BF16, kind="ExternalInput").ap()
    wv = nc.dram_tensor("wv", [D, W], BF16, kind="ExternalInput").ap()
    wo = nc.dram_tensor("wo", [W, D], BF16, kind="ExternalInput").ap()
    bq = nc.dram_tensor("bq", [1, W], BF16, kind="ExternalInput").ap()
    bk = nc.dram_tensor("bk", [1, W], BF16, kind="ExternalInput").ap()
    bv = nc.dram_tensor("bv", [1, W], BF16, kind="ExternalInput").ap()
    out = nc.dram_tensor("out", [D, T], F32, kind="ExternalOutput").ap()

    Exp = mybir.ActivationFunctionType.Exp

    with tile.TileContext(nc) as tc:
        with (
            tc.tile_pool(name="persist", bufs=1) as persist,
            tc.tile_pool(name="xpool", bufs=3) as xpool,
            tc.tile_pool(name="upool", bufs=4) as upool,
            tc.tile_pool(name="rpool", bufs=2) as rpool,
            tc.tile_pool(name="opool", bufs=4) as opool,
        ):
            # ---- constants ----
            ones_row = persist.tile([1, 512], BF16, tag="ones_row")
            nc.vector.memset(ones_row, 1.0)
            ones128 = persist.tile([1, 128], BF16, tag="ones128")
            nc.vector.memset(ones128, 1.0)
            # K=1 broadcast matmul stationary: ones row at partition 64
            bcast1 = persist.tile([65, 64], F32, tag="bcast1")
            nc.vector.memset(bcast1[64:65, :], 1.0)

            # ---- weights / biases ----
            wq_sb = persist.tile([128, 8, W], BF16, tag="wq")
            nc.sync.dma_start(out=wq_sb, in_=wq.rearrange("(c p) j -> p c j", p=128))
            wk_sb = persist.tile([128, 8, W], BF16, tag="wk")
            nc.sync.dma_start(out=wk_sb, in_=wk.rearrange("(c p) j -> p c j", p=128))
            wv_sb = persist.tile([128, 8, W], BF16, tag="wv")
            nc.sync.dma_start(out=wv_sb, in_=wv.rearrange("(c p) j -> p c j", p=128))
            wo_sb = persist.tile([128, 2, D], BF16, tag="wo")
            nc.sync.dma_start(out=wo_sb, in_=wo.rearrange("(c p) e -> p c e", p=128))
            bq_sb = persist.tile([1, W], BF16, tag="bq")
            nc.sync.dma_start(out=bq_sb, in_=bq)
            bk_sb = persist.tile([1, W], BF16, tag="bk")
            nc.sync.dma_start(out=bk_sb, in_=bk)
            bv_sb = persist.tile([1, W], BF16, tag="bv")
            nc.sync.dma_start(out=bv_sb, in_=bv)

            # ---- persistent activations ----
            qt_sb = persist.tile([128, 2, T], BF16, tag="qt")   # QT [j, t]
            kt_sb = persist.tile([128, 2, T], BF16, tag="kt")   # KT [j, t]
            # V augmented with ones column per head: [k, kt, h, 0:64]=V, [..64]=1
            vaug_sb = persist.tile([128, 16, HPC, Hd + 1], BF16, tag="vaug")
            nc.vector.memset(vaug_sb[:, :, :, 64:65], 1.0)
            otn_sb = persist.tile([128, 2, T], BF16, tag="otn")  # normalized O.T
            # raw [O.T ; denom] per block b2 = (pr*4+qt)*2 + m
            oraw_sb = persist.tile([65, 16, 512], F32, tag="oraw")

            # ================= Phase A: projections =================
            with tc.tile_pool(name="psA", bufs=8, space="PSUM") as psA:
                def qk_proj(x_dram, w_sb, b_sb, dst):
                    ps = [psA.tile([128, 512], F32, tag="proj", name=f"proj{i}")
                          for i in range(8)]
                    for c in range(8):
                        xc = xpool.tile([128, T], BF16, tag="x", name="xc")
                        nc.gpsimd.dma_start(out=xc,
                                            in_=x_dram[c * 128:(c + 1) * 128, :])
                        for jt in range(2):
                            for tt in range(4):
                                nc.tensor.matmul(
                                    ps[jt * 4 + tt],
                                    lhsT=w_sb[:, c, jt * 128:(jt + 1) * 128],
                                    rhs=xc[:, tt * 512:(tt + 1) * 512],
                                    start=(c == 0), stop=False,
                                )
                    for jt in range(2):
                        for tt in range(4):
                            p = ps[jt * 4 + tt]
                            nc.tensor.matmul(
                                p, lhsT=b_sb[:, jt * 128:(jt + 1) * 128],
                                rhs=ones_row, start=False, stop=True,
                            )
                            nc.vector.tensor_copy(
                                dst[:, jt, tt * 512:(tt + 1) * 512], p)

                def v_proj():
                    # V -> [t, j] layout (2 waves of 8 t-tiles)
                    for w in range(2):
                        ps = [psA.tile([128, W], F32, tag="proj",
                                       name=f"proj{i}") for i in range(8)]
                        for c in range(8):
                            xc = xpool.tile([128, T], BF16, tag="x", name="xc")
                            nc.gpsimd.dma_start(
                                out=xc, in_=xv[c * 128:(c + 1) * 128, :])
                            for i in range(8):
                                tt = w * 8 + i
                                nc.tensor.matmul(
                                    ps[i],
                                    lhsT=xc[:, tt * 128:(tt + 1) * 128],
                                    rhs=wv_sb[:, c, :],
                                    start=(c == 0), stop=False,
                                )
                        for i in range(8):
                            tt = w * 8 + i
                            nc.tensor.matmul(ps[i], lhsT=ones128, rhs=bv_sb,
                                             start=False, stop=True)
                            nc.vector.tensor_copy(
                                vaug_sb[:, tt, :, 0:64],
                                ps[i].rearrange("p (h d) -> p h d", h=HPC))

                qk_proj(xq, wq_sb, bq_sb, qt_sb)
                v_proj()
                qk_proj(xk, wk_sb, bk_sb, kt_sb)

            # ====== Phase B/D + fused normalization/output projection ======
            with tc.tile_pool(name="psB", bufs=1, space="PSUM") as psB:
                recips = {}

                def attn_block(pr, qt):
                    qsl = slice(qt * 512, (qt + 1) * 512)
                    o_psA = psB.tile([65, 512], F32, tag="oA", bufs=1,
                                     name="o_psA")
                    o_psB = psB.tile([65, 512], F32, tag="oB", bufs=1,
                                     name="o_psB")
                    us = []
                    for kt in range(17):
                        if kt < 16:
                            s_big = psB.tile([128, 2, 512], F32, tag="s",
                                             bufs=2, name="s_big")
                            for m in range(2):
                                po = 64 * m
                                nc.tensor.matmul(
                                    s_big[:, m, :],
                                    lhsT=kt_sb[po:po + 64, pr,
                                               kt * 128:(kt + 1) * 128],
                                    rhs=qt_sb[po:po + 64, pr, qsl],
                                    start=True, stop=True,
                                )
                            u_big = upool.tile([128, 2, 512], BF16, tag="u",
                                               name="u_big")
                            nc.scalar.activation(u_big, s_big, Exp, scale=SCALE)
                            us.append(u_big)
                        if kt >= 1:
                            for m, o_ps in ((0, o_psA), (1, o_psB)):
                                h = 2 * pr + m
                                nc.tensor.matmul(
                                    o_ps,
                                    lhsT=vaug_sb[:, kt - 1, h, :],
                                    rhs=us[kt - 1][:, m, :],
                                    start=(kt == 1), stop=(kt == 16),
                                )
                    # stage raw [O.T ; denom] to SBUF, then reciprocal of the
                    # denominator row (partition 64) on the DVE, off-path.
                    for m, o_ps in ((0, o_psA), (1, o_psB)):
                        b2 = (pr * 4 + qt) * 2 + m
                        nc.vector.tensor_copy(oraw_sb[:, b2, :], o_ps)
                        rt = rpool.tile([65, 512], F32, tag="rt", bufs=4,
                                        name="rt")
                        nc.vector.reciprocal(rt[64:65, :],
                                             oraw_sb[64:65, b2, :])
                        recips[b2] = rt

                def norm_and_proj(qt):
                    # normalize O.T for q tile qt and run its output projection
                    qsl = slice(qt * 512, (qt + 1) * 512)
                    for pr in range(2):
                        for m in range(2):
                            b2 = (pr * 4 + qt) * 2 + m
                            rb_ps = psB.tile([64, 512], F32, tag="rb", bufs=1,
                                             name="rb_ps")
                            nc.tensor.matmul(
                                rb_ps, lhsT=bcast1[64:65, :],
                                rhs=recips[b2][64:65, :],
                                start=True, stop=True)
                            rb_sb = rpool.tile([64, 512], F32, tag="rbs",
                                               name="rb_sb")
                            nc.vector.tensor_copy(rb_sb, rb_ps)
                            if m == 0:
                                nc.vector.tensor_mul(
                                    otn_sb[0:64, pr, qsl],
                                    oraw_sb[0:64, b2, :], rb_sb)
                            else:
                                otnB = rpool.tile([64, 512], BF16, tag="otnB",
                                                  name="otnB")
                                nc.vector.tensor_mul(
                                    otnB, oraw_sb[0:64, b2, :], rb_sb)
                                nc.sync.dma_start(
                                    out=otn_sb[64:128, pr, qsl], in_=otnB)
                    for et in range(8):
                        e_ps = psB.tile([128, 512], F32, tag="e", bufs=1,
                                        name="e_ps")
                        for jc in range(2):
                            nc.tensor.matmul(
                                e_ps,
                                lhsT=wo_sb[:, jc, et * 128:(et + 1) * 128],
                                rhs=otn_sb[:, jc, qsl],
                                start=(jc == 0), stop=(jc == 1),
                            )
                        stg = opool.tile([128, 512], F32, tag="ostg",
                                         name="stg")
                        nc.vector.tensor_copy(stg, e_ps)
                        nc.sync.dma_start(
                            out=out[et * 128:(et + 1) * 128, qsl], in_=stg)

                for qt in range(4):
                    attn_block(0, qt)
                    if qt >= 1:
                        norm_and_proj(qt - 1)
                    attn_block(1, qt)
                norm_and_proj(3)

    nc.finalize()
    return nc


_NC_CACHE = None


def _get_nc():
    global _NC_CACHE
    if _NC_CACHE is None:
        _NC_CACHE = build_nc()
    return _NC_CACHE


def make_in_maps(query, key, value, wq, bq, wk, bk, wv, bv, wo, bo):
    in_maps = []
    for c in range(N_CORES):
        b, hg = divmod(c, HPC)
        sl = slice(hg * W, (hg + 1) * W)
        in_maps.append({
            "xq": np.ascontiguousarray(np.asarray(query[b]).T).astype(bf16),
            "xk": np.ascontiguousarray(np.asarray(key[b]).T).astype(bf16),
            "xv": np.ascontiguousarray(np.asarray(value[b]).T).astype(bf16),
            "wq": np.ascontiguousarray(np.asarray(wq)[sl].T).astype(bf16),
            "wk": np.ascontiguousarray(np.asarray(wk)[sl].T).astype(bf16),
            "wv": np.ascontiguousarray(np.asarray(wv)[sl].T).astype(bf16),
            "wo": np.ascontiguousarray(np.asarray(wo)[:, sl].T).astype(bf16),
            "bq": np.asarray(bq)[sl].reshape(1, W).astype(bf16),
            "bk": np.asarray(bk)[sl].reshape(1, W).astype(bf16),
            "bv": np.asarray(bv)[sl].reshape(1, W).astype(bf16),
        })
    return in_maps


def combine_outputs(outs, bo):
    full = np.zeros((B, T, D), np.float32)
    for c in range(N_CORES):
        b = c // HPC
        full[b] += outs[c].T
    full += np.asarray(bo, np.float32)[None, None, :]
    return full


def kernel(query, key, value, wq, bq, wk, bk, wv, bv, wo, bo):
    nc = _get_nc()
    in_maps = make_in_maps(query, key, value, wq, bq, wk, bk, wv, bv, wo, bo)
    res = run_bass_kernel_spmd(nc, in_maps, list(range(N_CORES)))
    outs = [np.asarray(res.results[c]["out"]) for c in range(N_CORES)]
    return combine_outputs(outs, bo)


# revision 11
# speedup vs baseline: 1.9005x; 1.0720x over previous
"""Multi-head attention (B=2, T=2048, D=1024, H=16) on 8 TRN2 NeuronCores.

Sharding: 2D (batch x head-group). Core c handles batch b = c // 4 and head
group hg = c % 4 (4 heads = 256 channels of the projected dim). Each core:
  1. Projects its batch's q/k/v against its 256-row weight slices -> QT/KT
     in [j, t] layout and V in [t, j] layout (bf16, fp32 PSUM accumulation).
     V is stored augmented with a ones column per head: [V_h | 1].
     Order Q, V, K so attention never stalls waiting for V.
  2. Per head pair, per 512-wide q tile: S.T = K_h @ Q_h.T (transposed
     scores), U = exp(S.T * scale) (no max subtraction: |S*scale| <= ~16,
     exp fits fp32 easily), then [O.T ; denom] += [V_h | 1].T @ U -- the
     softmax denominator rides the PV matmul for free as output row 64.
     The PV matmuls trail the score/exp stage by one k tile so the PE
     never waits on ScalarE (keeps the HAM clock at 2.4 GHz).
  3. Raw [O.T ; denom] is staged to SBUF; per-block reciprocals run on
     idle DVE cycles; normalization + the output projection for q tile
     qt-1 are woven into the middle of qt's blocks as PE filler.
  4. out_partial.T = woT_chunk.T @ O_norm.T  -> [1024, 2048] fp32.
Host sums the 4 head-group partials per batch, transposes, adds bo.

PSUM discipline: exactly one accumulation group per PSUM bank (hardware
start=True clears has_written bits bank-wide). Engine ops only start at
partition offsets {0, 32, 64, 96}; partition shifts (head m=1 belongs at
rows 64-127 of the stage-E operand but results sit at rows 0-64) use
small SBUF->SBUF DMAs.

All shapes are hardcoded for this problem. kernel() takes the full inputs
and returns the full [2, 2048, 1024] fp32 output.
"""

import numpy as np
import ml_dtypes

import concourse.bass as bass
import concourse.bacc as bacc
import concourse.mybir as mybir
import concourse.tile as tile
from concourse.bass_utils import run_bass_kernel_spmd

B, T, D, H, Hd = 2, 2048, 1024, 16, 64
HPC = 4          # heads per core
W = HPC * Hd     # 256 projected channels per core
SCALE = Hd ** -0.5
N_CORES = 8

BF16 = mybir.dt.bfloat16
F32 = mybir.dt.float32
bf16 = ml_dtypes.bfloat16


def build_nc():
    nc = bacc.Bacc("TRN2", target_bir_lowering=False, debug=False)

    xq = nc.dram_tensor("xq", [D, T], BF16, kind="ExternalInput").ap()
    xk = nc.dram_tensor("xk", [D, T], BF16, kind="ExternalInput").ap()
    xv = nc.dram_tensor("xv", [D, T], BF16, kind="ExternalInput").ap()
    wq = nc.dram_tensor("wq", [D, W], BF16, kind="ExternalInput").ap()
    wk = nc.dram_tensor("wk", [D, W], BF16, kind="ExternalInput").ap()
    wv = nc.dram_tensor("wv", [D, W], BF16, kind="ExternalInput").ap()
    wo = nc.dram_tensor("wo", [W, D], BF16, kind="ExternalInput").ap()
    bq = nc.dram_tensor("bq", [1, W], BF16, kind="ExternalInput").ap()
    bk = nc.dram_tensor("bk", [1, W], BF16, kind="ExternalInput").ap()
    bv = nc.dram_tensor("bv", [1, W], BF16, kind="ExternalInput").ap()
    out = nc.dram_tensor("out", [D, T], F32, kind="ExternalOutput").ap()

    Exp = mybir.ActivationFunctionType.Exp

    with tile.TileContext(nc) as tc:
        with (
            tc.tile_pool(name="persist", bufs=1) as persist,
            tc.tile_pool(name="xpool", bufs=8) as xpool,
            tc.tile_pool(name="upool", bufs=4) as upool,
            tc.tile_pool(name="rpool", bufs=2) as rpool,
            tc.tile_pool(name="opool", bufs=4) as opool,
        ):
            # ---- constants ----
            ones_row = persist.tile([1, 512], BF16, tag="ones_row")
            nc.vector.memset(ones_row, 1.0)
            ones128 = persist.tile([1, 128], BF16, tag="ones128")
            nc.vector.memset(ones128, 1.0)
            # K=1 broadcast matmul stationary: ones row at partition 64
            bcast1 = persist.tile([65, 64], F32, tag="bcast1")
            nc.vector.memset(bcast1[64:65, :], 1.0)

            # ---- weights / biases ----
            wq_sb = persist.tile([128, 8, W], BF16, tag="wq")
            nc.sync.dma_start(out=wq_sb, in_=wq.rearrange("(c p) j -> p c j", p=128))
            wk_sb = persist.tile([128, 8, W], BF16, tag="wk")
            nc.sync.dma_start(out=wk_sb, in_=wk.rearrange("(c p) j -> p c j", p=128))
            wv_sb = persist.tile([128, 8, W], BF16, tag="wv")
            nc.sync.dma_start(out=wv_sb, in_=wv.rearrange("(c p) j -> p c j", p=128))
            wo_sb = persist.tile([128, 2, D], BF16, tag="wo")
            nc.sync.dma_start(out=wo_sb, in_=wo.rearrange("(c p) e -> p c e", p=128))
            bq_sb = persist.tile([1, W], BF16, tag="bq")
            nc.sync.dma_start(out=bq_sb, in_=bq)
            bk_sb = persist.tile([1, W], BF16, tag="bk")
            nc.sync.dma_start(out=bk_sb, in_=bk)
            bv_sb = persist.tile([1, W], BF16, tag="bv")
            nc.sync.dma_start(out=bv_sb, in_=bv)

            # ---- persistent activations ----
            qt_sb = persist.tile([128, 2, T], BF16, tag="qt")   # QT [j, t]
            kt_sb = persist.tile([128, 2, T], BF16, tag="kt")   # KT [j, t]
            # V augmented with ones column per head: [k, kt, h, 0:64]=V, [..64]=1
            vaug_sb = persist.tile([128, 16, HPC, Hd + 1], BF16, tag="vaug")
            nc.vector.memset(vaug_sb[:, :, :, 64:65], 1.0)
            otn_sb = persist.tile([128, 2, T], BF16, tag="otn")  # normalized O.T
            # raw [O.T ; denom] per block b2 = (pr*4+qt)*2 + m
            oraw_sb = persist.tile([65, 16, 512], F32, tag="oraw")

            # ================= Phase A: projections =================
            with tc.tile_pool(name="psA", bufs=8, space="PSUM") as psA:
                def qk_proj(x_dram, w_sb, b_sb, dst):
                    ps = [psA.tile([128, 512], F32, tag="proj", name=f"proj{i}")
                          for i in range(8)]
                    for c in range(8):
                        xc = xpool.tile([128, T], BF16, tag="x", name="xc")
                        nc.gpsimd.dma_start(out=xc,
                                            in_=x_dram[c * 128:(c + 1) * 128, :])
                        for jt in range(2):
                            for tt in range(4):
                                nc.tensor.matmul(
                                    ps[jt * 4 + tt],
                                    lhsT=w_sb[:, c, jt * 128:(jt + 1) * 128],
                                    rhs=xc[:, tt * 512:(tt + 1) * 512],
                                    start=(c == 0), stop=False,
                                )
                    for jt in range(2):
                        for tt in range(4):
                            p = ps[jt * 4 + tt]
                            nc.tensor.matmul(
                                p, lhsT=b_sb[:, jt * 128:(jt + 1) * 128],
                                rhs=ones_row, start=False, stop=True,
                            )
                            nc.vector.tensor_copy(
                                dst[:, jt, tt * 512:(tt + 1) * 512], p)

                def v_proj():
                    # V -> [t, j] layout (2 waves of 8 t-tiles)
                    for w in range(2):
                        ps = [psA.tile([128, W], F32, tag="proj",
                                       name=f"proj{i}") for i in range(8)]
                        for c in range(8):
                            xc = xpool.tile([128, T], BF16, tag="x", name="xc")
                            nc.gpsimd.dma_start(
                                out=xc, in_=xv[c * 128:(c + 1) * 128, :])
                            for i in range(8):
                                tt = w * 8 + i
                                nc.tensor.matmul(
                                    ps[i],
                                    lhsT=xc[:, tt * 128:(tt + 1) * 128],
                                    rhs=wv_sb[:, c, :],
                                    start=(c == 0), stop=False,
                                )
                        for i in range(8):
                            tt = w * 8 + i
                            nc.tensor.matmul(ps[i], lhsT=ones128, rhs=bv_sb,
                                             start=False, stop=True)
                            nc.vector.tensor_copy(
                                vaug_sb[:, tt, :, 0:64],
                                ps[i].rearrange("p (h d) -> p h d", h=HPC))

                qk_proj(xq, wq_sb, bq_sb, qt_sb)
                v_proj()
                qk_proj(xk, wk_sb, bk_sb, kt_sb)

            # ====== Phase B/D + fused normalization/output projection ======
            with tc.tile_pool(name="psB", bufs=1, space="PSUM") as psB:
                recips = {}

                def attn_block(pr, qt):
                    qsl = slice(qt * 512, (qt + 1) * 512)
                    o_psA = psB.tile([65, 512], F32, tag="oA", bufs=1,
                                     name="o_psA")
                    o_psB = psB.tile([65, 512], F32, tag="oB", bufs=1,
                                     name="o_psB")
                    us = []
                    for kt in range(17):
                        if kt < 16:
                            s_big = psB.tile([128, 2, 512], F32, tag="s",
                                             bufs=2, name="s_big")
                            for m in range(2):
                                po = 64 * m
                                nc.tensor.matmul(
                                    s_big[:, m, :],
                                    lhsT=kt_sb[po:po + 64, pr,
                                               kt * 128:(kt + 1) * 128],
                                    rhs=qt_sb[po:po + 64, pr, qsl],
                                    start=True, stop=True,
                                )
                            u_big = upool.tile([128, 2, 512], BF16, tag="u",
                                               name="u_big")
                            nc.scalar.activation(u_big, s_big, Exp, scale=SCALE)
                            us.append(u_big)
                        if kt >= 1:
                            for m, o_ps in ((0, o_psA), (1, o_psB)):
                                h = 2 * pr + m
                                nc.tensor.matmul(
                                    o_ps,
                                    lhsT=vaug_sb[:, kt - 1, h, :],
                                    rhs=us[kt - 1][:, m, :],
                                    start=(kt == 1), stop=(kt == 16),
                                )
                    # stage raw [O.T ; denom] to SBUF, then reciprocal of the
                    # denominator row (partition 64) on the DVE, off-path.
                    for m, o_ps in ((0, o_psA), (1, o_psB)):
                        b2 = (pr * 4 + qt) * 2 + m
                        nc.vector.tensor_copy(oraw_sb[:, b2, :], o_ps)
                        rt = rpool.tile([65, 512], F32, tag="rt", bufs=4,
                                        name="rt")
                        nc.vector.reciprocal(rt[64:65, :],
                                             oraw_sb[64:65, b2, :])
                        recips[b2] = rt

                def norm_and_proj(qt):
                    # normalize O.T for q tile qt and run its output projection
                    qsl = slice(qt * 512, (qt + 1) * 512)
                    for pr in range(2):
                        for m in range(2):
                            b2 = (pr * 4 + qt) * 2 + m
                            rb_ps = psB.tile([64, 512], F32, tag="rb", bufs=1,
                                             name="rb_ps")
                            nc.tensor.matmul(
                                rb_ps, lhsT=bcast1[64:65, :],
                                rhs=recips[b2][64:65, :],
                                start=True, stop=True)
                            rb_sb = rpool.tile([64, 512], F32, tag="rbs",
                                               name="rb_sb")
                            nc.vector.tensor_copy(rb_sb, rb_ps)
                            if m == 0:
                                nc.vector.tensor_mul(
                                    otn_sb[0:64, pr, qsl],
                                    oraw_sb[0:64, b2, :], rb_sb)
                            else:
                                otnB = rpool.tile([64, 512], BF16, tag="otnB",
                                                  name="otnB")
                                nc.vector.tensor_mul(
                                    otnB, oraw_sb[0:64, b2, :], rb_sb)
                                nc.sync.dma_start(
                                    out=otn_sb[64:128, pr, qsl], in_=otnB)
                    for et in range(8):
                        e_ps = psB.tile([128, 512], F32, tag="e", bufs=1,
                                        name="e_ps")
                        for jc in range(2):
                            nc.tensor.matmul(
                                e_ps,
                                lhsT=wo_sb[:, jc, et * 128:(et + 1) * 128],
                                rhs=otn_sb[:, jc, qsl],
                                start=(jc == 0), stop=(jc == 1),
                            )
                        stg = opool.tile([128, 512], F32, tag="ostg",
                                         name="stg")
                        nc.vector.tensor_copy(stg, e_ps)
                        nc.sync.dma_start(
                            out=out[et * 128:(et + 1) * 128, qsl], in_=stg)

                for qt in range(4):
                    attn_block(0, qt)
                    if qt >= 1:
                        norm_and_proj(qt - 1)
                    attn_block(1, qt)
                norm_and_proj(3)

    nc.finalize()
    return nc


_NC_CACHE = None


def _get_nc():
    global _NC_CACHE
    if _NC_CACHE is None:
        _NC_CACHE = build_nc()
    return _NC_CACHE


def make_in_maps(query, key, value, wq, bq, wk, bk, wv, bv, wo, bo):
    in_maps = []
    for c in range(N_CORES):
        b, hg = divmod(c, HPC)
        sl = slice(hg * W, (hg + 1) * W)
        in_maps.append({
            "xq": np.ascontiguousarray(np.asarray(query[b]).T).astype(bf16),
            "xk": np.ascontiguousarray(np.asarray(key[b]).T).astype(bf16),
            "xv": np.ascontiguousarray(np.asarray(value[b]).T).astype(bf16),
            "wq": np.ascontiguousarray(np.asarray(wq)[sl].T).astype(bf16),
            "wk": np.ascontiguousarray(np.asarray(wk)[sl].T).astype(bf16),
            "wv": np.ascontiguousarray(np.asarray(wv)[sl].T).astype(bf16),
            "wo": np.ascontiguousarray(np.asarray(wo)[:, sl].T).astype(bf16),
            "bq": np.asarray(bq)[sl].reshape(1, W).astype(bf16),
            "bk": np.asarray(bk)[sl].reshape(1, W).astype(bf16),
            "bv": np.asarray(bv)[sl].reshape(1, W).astype(bf16),
        })
    return in_maps


def combine_outputs(outs, bo):
    full = np.zeros((B, T, D), np.float32)
    for c in range(N_CORES):
        b = c // HPC
        full[b] += outs[c].T
    full += np.asarray(bo, np.float32)[None, None, :]
    return full


def kernel(query, key, value, wq, bq, wk, bk, wv, bv, wo, bo):
    nc = _get_nc()
    in_maps = make_in_maps(query, key, value, wq, bq, wk, bk, wv, bv, wo, bo)
    res = run_bass_kernel_spmd(nc, in_maps, list(range(N_CORES)))
    outs = [np.asarray(res.results[c]["out"]) for c in range(N_CORES)]
    return combine_outputs(outs, bo)


# revision 12
# speedup vs baseline: 1.9165x; 1.0084x over previous
"""Multi-head attention (B=2, T=2048, D=1024, H=16) on 8 TRN2 NeuronCores.

Sharding: 2D (batch x head-group). Core c handles batch b = c // 4 and head
group hg = c % 4 (4 heads = 256 channels of the projected dim). Each core:
  1. Projects its batch's q/k/v against its 256-row weight slices -> QT/KT
     in [j, t] layout and V in [t, j] layout (bf16, fp32 PSUM accumulation).
     V is stored augmented with a ones column per head: [V_h | 1].
     Order Q, V, K so attention never stalls waiting for V.
  2. Per head pair, per 512-wide q tile: S.T = K_h @ Q_h.T (transposed
     scores), U = exp(S.T * scale) (no max subtraction: |S*scale| <= ~16,
     exp fits fp32 easily), then [O.T ; denom] += [V_h | 1].T @ U -- the
     softmax denominator rides the PV matmul for free as output row 64.
     The PV matmuls trail the score/exp stage by one k tile so the PE
     never waits on ScalarE (keeps the HAM clock at 2.4 GHz).
  3. Raw [O.T ; denom] is staged to SBUF; per-block reciprocals run on
     idle DVE cycles; normalization + the output projection for q tile
     qt-1 are woven into the middle of qt's blocks as PE filler.
  4. out_partial.T = woT_chunk.T @ O_norm.T  -> [1024, 2048] fp32.
Host sums the 4 head-group partials per batch, transposes, adds bo.

PSUM discipline: exactly one accumulation group per PSUM bank (hardware
start=True clears has_written bits bank-wide). Engine ops only start at
partition offsets {0, 32, 64, 96}; partition shifts (head m=1 belongs at
rows 64-127 of the stage-E operand but results sit at rows 0-64) use
small SBUF->SBUF DMAs.

All shapes are hardcoded for this problem. kernel() takes the full inputs
and returns the full [2, 2048, 1024] fp32 output.
"""

import numpy as np
import ml_dtypes

import concourse.bass as bass
import concourse.bacc as bacc
import concourse.mybir as mybir
import concourse.tile as tile
from concourse.bass_utils import run_bass_kernel_spmd

B, T, D, H, Hd = 2, 2048, 1024, 16, 64
HPC = 4          # heads per core
W = HPC * Hd     # 256 projected channels per core
SCALE = Hd ** -0.5
N_CORES = 8

BF16 = mybir.dt.bfloat16
F32 = mybir.dt.float32
bf16 = ml_dtypes.bfloat16


def build_nc():
    nc = bacc.Bacc("TRN2", target_bir_lowering=False, debug=False)

    xq = nc.dram_tensor("xq", [D, T], BF16, kind="ExternalInput").ap()
    xk = nc.dram_tensor("xk", [D, T], BF16, kind="ExternalInput").ap()
    xv = nc.dram_tensor("xv", [D, T], BF16, kind="ExternalInput").ap()
    wq = nc.dram_tensor("wq", [D, W], BF16, kind="ExternalInput").ap()
    wk = nc.dram_tensor("wk", [D, W], BF16, kind="ExternalInput").ap()
    wv = nc.dram_tensor("wv", [D, W], BF16, kind="ExternalInput").ap()
    wo = nc.dram_tensor("wo", [W, D], BF16, kind="ExternalInput").ap()
    bq = nc.dram_tensor("bq", [1, W], BF16, kind="ExternalInput").ap()
    bk = nc.dram_tensor("bk", [1, W], BF16, kind="ExternalInput").ap()
    bv = nc.dram_tensor("bv", [1, W], BF16, kind="ExternalInput").ap()
    out = nc.dram_tensor("out", [D, T], F32, kind="ExternalOutput").ap()

    Exp = mybir.ActivationFunctionType.Exp

    with tile.TileContext(nc) as tc:
        with (
            tc.tile_pool(name="persist", bufs=1) as persist,
            tc.tile_pool(name="xpool", bufs=8) as xpool,
            tc.tile_pool(name="upool", bufs=8) as upool,
            tc.tile_pool(name="rpool", bufs=2) as rpool,
            tc.tile_pool(name="opool", bufs=4) as opool,
        ):
            # ---- constants ----
            ones_row = persist.tile([1, 512], BF16, tag="ones_row")
            nc.vector.memset(ones_row, 1.0)
            ones128 = persist.tile([1, 128], BF16, tag="ones128")
            nc.vector.memset(ones128, 1.0)
            # K=1 broadcast matmul stationary: ones row at partition 64
            bcast1 = persist.tile([65, 64], F32, tag="bcast1")
            nc.vector.memset(bcast1[64:65, :], 1.0)

            # ---- weights / biases ----
            wq_sb = persist.tile([128, 8, W], BF16, tag="wq")
            nc.sync.dma_start(out=wq_sb, in_=wq.rearrange("(c p) j -> p c j", p=128))
            wk_sb = persist.tile([128, 8, W], BF16, tag="wk")
            nc.sync.dma_start(out=wk_sb, in_=wk.rearrange("(c p) j -> p c j", p=128))
            wv_sb = persist.tile([128, 8, W], BF16, tag="wv")
            nc.sync.dma_start(out=wv_sb, in_=wv.rearrange("(c p) j -> p c j", p=128))
            wo_sb = persist.tile([128, 2, D], BF16, tag="wo")
            nc.sync.dma_start(out=wo_sb, in_=wo.rearrange("(c p) e -> p c e", p=128))
            bq_sb = persist.tile([1, W], BF16, tag="bq")
            nc.sync.dma_start(out=bq_sb, in_=bq)
            bk_sb = persist.tile([1, W], BF16, tag="bk")
            nc.sync.dma_start(out=bk_sb, in_=bk)
            bv_sb = persist.tile([1, W], BF16, tag="bv")
            nc.sync.dma_start(out=bv_sb, in_=bv)

            # ---- persistent activations ----
            qt_sb = persist.tile([128, 2, T], BF16, tag="qt")   # QT [j, t]
            kt_sb = persist.tile([128, 2, T], BF16, tag="kt")   # KT [j, t]
            # V augmented with ones column per head: [k, kt, h, 0:64]=V, [..64]=1
            vaug_sb = persist.tile([128, 16, HPC, Hd + 1], BF16, tag="vaug")
            nc.vector.memset(vaug_sb[:, :, :, 64:65], 1.0)
            otn_sb = persist.tile([128, 2, T], BF16, tag="otn")  # normalized O.T
            # raw [O.T ; denom] per block b2 = (pr*4+qt)*2 + m
            oraw_sb = persist.tile([65, 16, 512], F32, tag="oraw")

            # ================= Phase A: projections =================
            with tc.tile_pool(name="psA", bufs=8, space="PSUM") as psA:
                def qk_proj(x_dram, w_sb, b_sb, dst):
                    ps = [psA.tile([128, 512], F32, tag="proj", name=f"proj{i}")
                          for i in range(8)]
                    for c in range(8):
                        xc = xpool.tile([128, T], BF16, tag="x", name="xc")
                        nc.gpsimd.dma_start(out=xc,
                                            in_=x_dram[c * 128:(c + 1) * 128, :])
                        for jt in range(2):
                            for tt in range(4):
                                nc.tensor.matmul(
                                    ps[jt * 4 + tt],
                                    lhsT=w_sb[:, c, jt * 128:(jt + 1) * 128],
                                    rhs=xc[:, tt * 512:(tt + 1) * 512],
                                    start=(c == 0), stop=False,
                                )
                    for jt in range(2):
                        for tt in range(4):
                            p = ps[jt * 4 + tt]
                            nc.tensor.matmul(
                                p, lhsT=b_sb[:, jt * 128:(jt + 1) * 128],
                                rhs=ones_row, start=False, stop=True,
                            )
                            nc.vector.tensor_copy(
                                dst[:, jt, tt * 512:(tt + 1) * 512], p)

                def v_proj():
                    # V -> [t, j] layout (2 waves of 8 t-tiles)
                    for w in range(2):
                        ps = [psA.tile([128, W], F32, tag="proj",
                                       name=f"proj{i}") for i in range(8)]
                        for c in range(8):
                            xc = xpool.tile([128, T], BF16, tag="x", name="xc")
                            nc.gpsimd.dma_start(
                                out=xc, in_=xv[c * 128:(c + 1) * 128, :])
                            for i in range(8):
                                tt = w * 8 + i
                                nc.tensor.matmul(
                                    ps[i],
                                    lhsT=xc[:, tt * 128:(tt + 1) * 128],
                                    rhs=wv_sb[:, c, :],
                                    start=(c == 0), stop=False,
                                )
                        for i in range(8):
                            tt = w * 8 + i
                            nc.tensor.matmul(ps[i], lhsT=ones128, rhs=bv_sb,
                                             start=False, stop=True)
                            nc.vector.tensor_copy(
                                vaug_sb[:, tt, :, 0:64],
                                ps[i].rearrange("p (h d) -> p h d", h=HPC))

                # K then Q so the attention score matmuls + exp can begin
                # while the V projection still runs (V only gates the PV
                # matmuls, which trail by a k tile anyway).
                qk_proj(xk, wk_sb, bk_sb, kt_sb)
                qk_proj(xq, wq_sb, bq_sb, qt_sb)
                v_proj()

            # ====== Phase B/D + fused normalization/output projection ======
            with tc.tile_pool(name="psB", bufs=1, space="PSUM") as psB:
                recips = {}

                def attn_block(pr, qt):
                    qsl = slice(qt * 512, (qt + 1) * 512)
                    o_psA = psB.tile([65, 512], F32, tag="oA", bufs=1,
                                     name="o_psA")
                    o_psB = psB.tile([65, 512], F32, tag="oB", bufs=1,
                                     name="o_psB")
                    us = []
                    for kt in range(17):
                        if kt < 16:
                            s_big = psB.tile([128, 2, 512], F32, tag="s",
                                             bufs=2, name="s_big")
                            for m in range(2):
                                po = 64 * m
                                nc.tensor.matmul(
                                    s_big[:, m, :],
                                    lhsT=kt_sb[po:po + 64, pr,
                                               kt * 128:(kt + 1) * 128],
                                    rhs=qt_sb[po:po + 64, pr, qsl],
                                    start=True, stop=True,
                                )
                            u_big = upool.tile([128, 2, 512], BF16, tag="u",
                                               name="u_big")
                            nc.scalar.activation(u_big, s_big, Exp, scale=SCALE)
                            us.append(u_big)
                        if kt >= 1:
                            for m, o_ps in ((0, o_psA), (1, o_psB)):
                                h = 2 * pr + m
                                nc.tensor.matmul(
                                    o_ps,
                                    lhsT=vaug_sb[:, kt - 1, h, :],
                                    rhs=us[kt - 1][:, m, :],
                                    start=(kt == 1), stop=(kt == 16),
                                )
                    # stage raw [O.T ; denom] to SBUF, then reciprocal of the
                    # denominator row (partition 64) on the DVE, off-path.
                    for m, o_ps in ((0, o_psA), (1, o_psB)):
                        b2 = (pr * 4 + qt) * 2 + m
                        nc.vector.tensor_copy(oraw_sb[:, b2, :], o_ps)
                        rt = rpool.tile([65, 512], F32, tag="rt", bufs=4,
                                        name="rt")
                        nc.vector.reciprocal(rt[64:65, :],
                                             oraw_sb[64:65, b2, :])
                        recips[b2] = rt

                def norm_and_proj(qt):
                    # normalize O.T for q tile qt and run its output projection
                    qsl = slice(qt * 512, (qt + 1) * 512)
                    for pr in range(2):
                        for m in range(2):
                            b2 = (pr * 4 + qt) * 2 + m
                            rb_ps = psB.tile([64, 512], F32, tag="rb", bufs=1,
                                             name="rb_ps")
                            nc.tensor.matmul(
                                rb_ps, lhsT=bcast1[64:65, :],
                                rhs=recips[b2][64:65, :],
                                start=True, stop=True)
                            rb_sb = rpool.tile([64, 512], F32, tag="rbs",
                                               name="rb_sb")
                            nc.vector.tensor_copy(rb_sb, rb_ps)
                            if m == 0:
                                nc.vector.tensor_mul(
                                    otn_sb[0:64, pr, qsl],
                                    oraw_sb[0:64, b2, :], rb_sb)
                            else:
                                otnB = rpool.tile([64, 512], BF16, tag="otnB",
                                                  name="otnB")
                                nc.vector.tensor_mul(
                                    otnB, oraw_sb[0:64, b2, :], rb_sb)
                                nc.sync.dma_start(
                                    out=otn_sb[64:128, pr, qsl], in_=otnB)
                    for et in range(8):
                        e_ps = psB.tile([128, 512], F32, tag="e", bufs=1,
                                        name="e_ps")
                        for jc in range(2):
                            nc.tensor.matmul(
                                e_ps,
                                lhsT=wo_sb[:, jc, et * 128:(et + 1) * 128],
                                rhs=otn_sb[:, jc, qsl],
                                start=(jc == 0), stop=(jc == 1),
                            )
                        stg = opool.tile([128, 512], F32, tag="ostg",
                                         name="stg")
                        nc.vector.tensor_copy(stg, e_ps)
                        nc.sync.dma_start(
                            out=out[et * 128:(et + 1) * 128, qsl], in_=stg)

                for qt in range(4):
                    attn_block(0, qt)
                    if qt >= 1:
                        norm_and_proj(qt - 1)
                    attn_block(1, qt)
                norm_and_proj(3)

    nc.finalize()
    return nc


_NC_CACHE = None


def _get_nc():
    global _NC_CACHE
    if _NC_CACHE is None:
        _NC_CACHE = build_nc()
    return _NC_CACHE


def make_in_maps(query, key, value, wq, bq, wk, bk, wv, bv, wo, bo):
    in_maps = []
    for c in range(N_CORES):
        b, hg = divmod(c, HPC)
        sl = slice(hg * W, (hg + 1) * W)
        in_maps.append({
            "xq": np.ascontiguousarray(np.asarray(query[b]).T).astype(bf16),
            "xk": np.ascontiguousarray(np.asarray(key[b]).T).astype(bf16),
            "xv": np.ascontiguousarray(np.asarray(value[b]).T).astype(bf16),
            "wq": np.ascontiguousarray(np.asarray(wq)[sl].T).astype(bf16),
            "wk": np.ascontiguousarray(np.asarray(wk)[sl].T).astype(bf16),
            "wv": np.ascontiguousarray(np.asarray(wv)[sl].T).astype(bf16),
            "wo": np.ascontiguousarray(np.asarray(wo)[:, sl].T).astype(bf16),
            "bq": np.asarray(bq)[sl].reshape(1, W).astype(bf16),
            "bk": np.asarray(bk)[sl].reshape(1, W).astype(bf16),
            "bv": np.asarray(bv)[sl].reshape(1, W).astype(bf16),
        })
    return in_maps


def combine_outputs(outs, bo):
    full = np.zeros((B, T, D), np.float32)
    for c in range(N_CORES):
        b = c // HPC
        full[b] += outs[c].T
    full += np.asarray(bo, np.float32)[None, None, :]
    return full


def kernel(query, key, value, wq, bq, wk, bk, wv, bv, wo, bo):
    nc = _get_nc()
    in_maps = make_in_maps(query, key, value, wq, bq, wk, bk, wv, bv, wo, bo)
    res = run_bass_kernel_spmd(nc, in_maps, list(range(N_CORES)))
    outs = [np.asarray(res.results[c]["out"]) for c in range(N_CORES)]
    return combine_outputs(outs, bo)
